# revision 13
# baseline (speedup 1.0000x reference)
"""YOLO-head decode (nms_detection) Bass kernel for 8 trn2 NeuronCores.

Reference computation per pyramid level p [S, S, 3, 85]:
  conf = p[...,0]
  x = (sigmoid(p[...,1]) + i) / S        (i = index along FIRST spatial axis)
  y = (sigmoid(p[...,2]) + j) / S
  w = exp(p[...,3]) * anchor_w           (anchor = pre_scale[dect]/416)
  h = exp(p[...,4]) * anchor_h
  lix = argmax(p[...,5:85])  (first-max tie-break)
  row = [x,y,w,h,lix,conf] * (conf > 0.5)
Output = concat over levels of rows, [681408, 6].

Sharding: each level split along its leading spatial axis into 8 row-shards
(104->13, 208->26, 416->52 rows per core). Decode is elementwise per cell, so
cores are fully independent; host concatenates the per-core outputs.

Layout: cells on partitions, K cells per partition per tile, padded per level
so all 128 partitions are used (small 1x128x32, middle 2x128x64, large
13x128x39; pad cells are zeros -> conf 0 -> masked, host drops them). Grid
x/y offsets and anchors are per-(tile, partition, k) constants computed on
host into small per-level tables (no per-cell aux tensor DMA).

argmax via value/index packing so one max-reduce yields both:
  ACT:  t   = Identity(v * 2^14 + 1.5*2^23)      -> fp32 magic-round, t = M + r
  DVE:  key = (t - M) + ((79-c)/128 - 0.3125)    -> r + (m-40)/128, m = 79-c
  DVE:  kmax = reduce_max(key)  (single InstTensorReduce over the 80 classes)
  ACT:  t2  = Identity(kmax + M)                 -> M + r*
  DVE:  e   = (t2 - M) - kmax = -(m*-40)/128 ;  lix = 128*e + 39 = c*
All steps are exact in fp32 for |v| < 8 (keys are integers r plus exact
multiples of 1/128, |r| <= 2^17), so the only deviation from jnp.argmax is
the 2^-14 quantization of logits (ties broken toward the first index, same
as argmax; measured rel-err contribution ~3.7e-3, well under the 2e-2 gate).

sigmoid is computed as 1/(1+exp(-x)) (ACT Exp with scale=-1 + Identity(+1),
DVE reciprocal_approx_fast) because no ACT table set holds both sigmoid and
exp -- a native Sigmoid would force two 1.3us table reloads per tile.

Outputs are written bf16 (halves output DMA; x/y/w/h/conf tolerate the 2^-9
rounding, lix <= 79 is exact in bf16) and converted to fp32 on host.
"""

import os
import sys
from contextlib import ExitStack

import numpy as np

for _p in ("/root/.axon_site/_ro/trn_rl_repo", "/opt/trn_rl_repo"):
    if os.path.isdir(_p) and _p not in sys.path:
        sys.path.append(_p)

import concourse.bacc as bacc
import concourse.bass as bass
import concourse.tile as tile
import concourse.mybir as mybir
from concourse.bass_utils import run_bass_kernel_spmd

F32 = mybir.dt.float32
BF16 = mybir.dt.bfloat16
Alu = mybir.AluOpType
Act = mybir.ActivationFunctionType
AxX = mybir.AxisListType.X

N_CORES = 8
MAGIC = 12582912.0  # 1.5 * 2^23: float + MAGIC - MAGIC == round-to-nearest-int
QS = 16384.0  # 2^14 logit quantization scale

# (name, S, rows_per_core, dect_size, P, K, T, A)
#   P*K*T >= rows*S*3 (pad), A = number of distinct anchor k-patterns over t
# Processing order large->small keeps the DMA-paced pipeline ramp short (the
# big tiles keep DVE fed while later tiles stream in); OUT_ORDER is the
# reference's concat order.
LEVELS = [
    ("large", 416, 52, 5, 128, 39, 13, 1),
    ("middle", 208, 26, 4, 128, 64, 2, 2),
    ("small", 104, 13, 3, 128, 32, 1, 1),
]
OUT_ORDER = ["small", "middle", "large"]

LAST_EXEC_NS = None
LAST_RESULTS = None

_prog_cache = {}


def _build_program():
    nc = bacc.Bacc(trn_type="TRN2")
    xins, csts, outs = {}, {}, {}
    for nm, S, rows, dect, P, K, T, A in LEVELS:
        Ncap = P * K * T
        W = (T + A) * 2 * K
        xins[nm] = nc.dram_tensor(f"x_{nm}", [Ncap, 85], F32, kind="ExternalInput")
        csts[nm] = nc.dram_tensor(f"c_{nm}", [128, W], F32, kind="ExternalInput")
        outs[nm] = nc.dram_tensor(f"o_{nm}", [Ncap, 6], BF16, kind="ExternalOutput")
    # cols 0:80 = (79-c)/128 - 0.3125 ; col 80 = MAGIC (activation bias AP)
    iot = nc.dram_tensor("iota", [128, 81], F32, kind="ExternalInput")

    with tile.TileContext(nc) as tc, ExitStack() as ctx:
        const = ctx.enter_context(tc.tile_pool(name="const", bufs=1))
        pin_p = ctx.enter_context(tc.tile_pool(name="pin", bufs=4))
        qt_p = ctx.enter_context(tc.tile_pool(name="qtp", bufs=2))
        km_p = ctx.enter_context(tc.tile_pool(name="kmp", bufs=2))
        sml_p = ctx.enter_context(tc.tile_pool(name="sml", bufs=3))
        out_p = ctx.enter_context(tc.tile_pool(name="outp", bufs=3))

        it = const.tile([128, 81], F32)
        nc.sync.dma_start(it[:], iot[:])
        ctiles = {}
        for nm, S, rows, dect, P, K, T, A in LEVELS:
            ct = const.tile([128, (T + A) * 2 * K], F32)
            nc.sync.dma_start(ct[:], csts[nm][:])
            ctiles[nm] = ct

        magic_ap = it[0:128, 80:81]
        for nm, S, rows, dect, P, K, T, A in LEVELS:
            inv = float(np.float32(1.0 / S))
            xv = xins[nm][:].rearrange("(t p k) c -> t p (k c)", p=P, k=K)
            ov = outs[nm][:].rearrange("(t p k) c -> t p (k c)", p=P, k=K)
            ct = ctiles[nm]
            iota_b = (
                it[0:P, 0:80]
                .rearrange("p (o c) -> p o c", o=1)
                .broadcast_to([P, K, 80])
            )
            for t in range(T):
                gxy_v = ct[0:P, t * 2 * K : (t + 1) * 2 * K].rearrange(
                    "p (k c) -> p k c", c=2
                )
                ao = (T + (t % A)) * 2 * K
                awh_v = ct[0:P, ao : ao + 2 * K].rearrange("p (k c) -> p k c", c=2)

                pin = pin_p.tile([P, K * 85], F32, tag="pin")
                nc.sync.dma_start(pin[:], xv[t])
                pv = pin[:].rearrange("p (k c) -> p k c", c=85)

                ot = out_p.tile([P, K * 6], BF16, tag="out")
                ovv = ot[:].rearrange("p (k c) -> p k c", c=6)

                # ACT: magic-round first (it gates DVE's big ops), then
                # exp(-txy) for sigmoid, exp(twh), conf copy
                qt = qt_p.tile([P, K * 80], F32, tag="qt")
                qv = qt[:].rearrange("p (k c) -> p k c", c=80)
                nc.scalar.activation(
                    qv, pv[:, :, 5:85], Act.Identity, bias=magic_ap, scale=QS
                )
                es = sml_p.tile([P, K * 2], F32, tag="es")
                esv = es[:].rearrange("p (k c) -> p k c", c=2)
                nc.scalar.activation(esv, pv[:, :, 1:3], Act.Exp, scale=-1.0)
                sp = sml_p.tile([P, K * 2], F32, tag="sp")
                spv = sp[:].rearrange("p (k c) -> p k c", c=2)
                nc.scalar.activation(spv, esv, Act.Identity, bias=1.0)
                ext = sml_p.tile([P, K * 2], F32, tag="exp")
                exv = ext[:].rearrange("p (k c) -> p k c", c=2)
                nc.scalar.activation(exv, pv[:, :, 3:5], Act.Exp)
                nc.scalar.copy(ovv[:, :, 5:6], pv[:, :, 0:1])

                # DVE: key = (t - M) + iota'   (in place over qt)
                nc.vector.scalar_tensor_tensor(qv, qv, -MAGIC, iota_b, Alu.add, Alu.add)

                # DVE: packed max over the 80 classes in one reduce
                km = km_p.tile([P, K], F32, tag="km")
                nc.vector.tensor_reduce(km[:], qv, axis=AxX, op=Alu.max)

                # DVE: t2 = M + r* (fp32 write rounds); e = (t2 - M) - kmax;
                # lix = 128*e + 39 (exact int)
                t2 = sml_p.tile([P, K], F32, tag="t2")
                nc.vector.tensor_scalar(t2[:], km[:], 1.0, MAGIC, Alu.mult, Alu.add)
                e = sml_p.tile([P, K], F32, tag="e")
                nc.vector.scalar_tensor_tensor(
                    e[:], t2[:], -MAGIC, km[:], Alu.add, Alu.subtract
                )
                ev = e[:].rearrange("p (k c) -> p k c", c=1)
                nc.vector.tensor_scalar(
                    ovv[:, :, 4:5], ev, 128.0, 39.0, Alu.mult, Alu.add
                )

                # DVE: sigmoid = 1/(1+exp(-x)); (x,y) = sig*inv + (gx,gy);
                # (w,h) = exp * (aw,ah); mask = conf > 0.5; zero failing rows
                sg = sml_p.tile([P, K * 2], F32, tag="sg")
                nc.vector.reciprocal_approx_fast(sg[:], sp[:])
                sgv = sg[:].rearrange("p (k c) -> p k c", c=2)
                nc.vector.scalar_tensor_tensor(
                    ovv[:, :, 0:2], sgv, inv, gxy_v, Alu.mult, Alu.add
                )
                nc.vector.scalar_tensor_tensor(
                    ovv[:, :, 2:4], exv, 1.0, awh_v, Alu.mult, Alu.mult
                )
                mk = sml_p.tile([P, K], BF16, tag="mk")
                nc.vector.tensor_scalar(mk[:], pv[:, :, 0], 0.5, None, Alu.is_gt)
                mk_b = mk[:].rearrange("p (k o) -> p k o", o=1).broadcast_to([P, K, 6])
                nc.vector.scalar_tensor_tensor(ovv, ovv, 1.0, mk_b, Alu.mult, Alu.mult)

                nc.sync.dma_start(ov[t], ot[:])
    nc.compile()
    return nc


def _get_program():
    if "nc" not in _prog_cache:
        _prog_cache["nc"] = _build_program()
    return _prog_cache["nc"]


def _make_cst(core, ps, S, rows, dect, P, K, T, A):
    """Per-level const table [128, (T+A)*2K]: per-tile (gx,gy) | awh variants."""
    inv = np.float32(1.0 / S)
    anc = (ps[dect] / np.float32(416.0)).astype(np.float32)  # [3, 2]
    cells = rows * S * 3
    Ncap = P * K * T
    base = core * cells
    g = base + np.minimum(np.arange(Ncap), cells - 1)
    arr = g.reshape(T, P, K)
    i = arr // (S * 3)
    j = (arr % (S * 3)) // 3
    aa = arr % 3
    cst = np.zeros((128, (T + A) * 2 * K), np.float32)
    blk = np.empty((P, K, 2), np.float32)
    for t in range(T):
        blk[..., 0] = i[t].astype(np.float32) * inv
        blk[..., 1] = j[t].astype(np.float32) * inv
        cst[:P, t * 2 * K : (t + 1) * 2 * K] = blk.reshape(P, 2 * K)
    for va in range(A):
        blk[..., 0] = anc[aa[va], 0]
        blk[..., 1] = anc[aa[va], 1]
        cst[:P, (T + va) * 2 * K : (T + va + 1) * 2 * K] = blk.reshape(P, 2 * K)
    return cst


def _make_in_maps(small, middle, large, pre_scale):
    arrs = {"small": small, "middle": middle, "large": large}
    ps = np.asarray(pre_scale, dtype=np.float32)
    iota = np.empty((128, 81), np.float32)
    iota[:, 0:80] = (79.0 - np.arange(80)).astype(np.float32) / np.float32(
        128.0
    ) - np.float32(0.3125)
    iota[:, 80] = MAGIC
    in_maps = []
    for c in range(N_CORES):
        m = {"iota": iota}
        for nm, S, rows, dect, P, K, T, A in LEVELS:
            cells = rows * S * 3
            Ncap = P * K * T
            x = np.asarray(arrs[nm][c * rows : (c + 1) * rows], dtype=np.float32)
            xp = np.zeros((Ncap, 85), np.float32)
            xp[:cells] = x.reshape(cells, 85)
            m[f"x_{nm}"] = xp
            m[f"c_{nm}"] = _make_cst(c, ps, S, rows, dect, P, K, T, A)
        in_maps.append(m)
    return in_maps


def kernel(small, middle, large, pre_scale):
    global LAST_EXEC_NS, LAST_RESULTS
    small = np.asarray(small, dtype=np.float32)
    middle = np.asarray(middle, dtype=np.float32)
    large = np.asarray(large, dtype=np.float32)
    in_maps = _make_in_maps(small, middle, large, pre_scale)
    nc = _get_program()
    res = run_bass_kernel_spmd(nc, in_maps, list(range(N_CORES)))
    LAST_EXEC_NS = res.exec_time_ns
    LAST_RESULTS = res
    by_name = {lv[0]: lv for lv in LEVELS}
    chunks = []
    for nm in OUT_ORDER:
        nm, S, rows, dect, P, K, T, A = by_name[nm]
        cells = rows * S * 3
        for c in range(N_CORES):
            o = np.asarray(res.results[c][f"o_{nm}"])[:cells]
            chunks.append(o.astype(np.float32))
    return np.concatenate(chunks, axis=0)


# revision 15
# speedup vs baseline: 1.0041x; 1.0041x over previous
"""YOLO-head decode (nms_detection) Bass kernel for 8 trn2 NeuronCores.

Reference computation per pyramid level p [S, S, 3, 85]:
  conf = p[...,0]
  x = (sigmoid(p[...,1]) + i) / S        (i = index along FIRST spatial axis)
  y = (sigmoid(p[...,2]) + j) / S
  w = exp(p[...,3]) * anchor_w           (anchor = pre_scale[dect]/416)
  h = exp(p[...,4]) * anchor_h
  lix = argmax(p[...,5:85])  (first-max tie-break)
  row = [x,y,w,h,lix,conf] * (conf > 0.5)
Output = concat over levels of rows, [681408, 6].

Sharding: each level split along its leading spatial axis into 8 row-shards
(104->13, 208->26, 416->52 rows per core). Decode is elementwise per cell, so
cores are fully independent; host concatenates the per-core outputs.

Layout: cells on partitions (always all 128), K cells per partition per tile
with a per-level list of tile widths. The large level starts with K=8/K=19
tiles so the first DVE work is gated by only ~350KB of DMA (short pipeline
ramp), then runs K=78 tiles to amortize per-instruction overheads. Levels are
padded (pad cells are zeros -> conf 0 -> masked; host drops them). Grid x/y
offsets and anchors are per-(tile, partition, k) host-computed const tables.

argmax via value/index packing so one max-reduce yields both:
  ACT:  t   = Identity(v * 2^14 + 1.5*2^23)      -> fp32 magic-round, t = M + r
  DVE:  key = (t - M) + ((79-c)/128 - 0.3125)    -> r + (m-40)/128, m = 79-c
  DVE:  kmax = reduce_max(key)  (single InstTensorReduce over the 80 classes)
  DVE:  t2  = kmax + M (fp32 write rounds) ; e = (t2 - M) - kmax = -(m*-40)/128
  DVE:  lix = 128*e + 39 = c*
All steps are exact in fp32 for |v| < 8 (keys are integers r plus exact
multiples of 1/128, |r| <= 2^17), so the only deviation from jnp.argmax is
the 2^-14 quantization of logits (ties broken toward the first index, same
as argmax; measured rel-err contribution ~3.7e-3, well under the 2e-2 gate).

sigmoid is computed as 1/(1+exp(-x)) (ACT Exp with scale=-1 + Identity(+1),
DVE reciprocal_approx_fast) because no ACT table set holds both sigmoid and
exp -- a native Sigmoid would force two 1.3us table reloads per tile.

Outputs are written bf16 (halves output DMA; x/y/w/h/conf tolerate the 2^-9
rounding, lix <= 79 is exact in bf16) and converted to fp32 on host.
"""

import os
import sys
from contextlib import ExitStack

import numpy as np

for _p in ("/root/.axon_site/_ro/trn_rl_repo", "/opt/trn_rl_repo"):
    if os.path.isdir(_p) and _p not in sys.path:
        sys.path.append(_p)

import concourse.bacc as bacc
import concourse.bass as bass
import concourse.tile as tile
import concourse.mybir as mybir
from concourse.bass_utils import run_bass_kernel_spmd

F32 = mybir.dt.float32
BF16 = mybir.dt.bfloat16
Alu = mybir.AluOpType
Act = mybir.ActivationFunctionType
AxX = mybir.AxisListType.X

N_CORES = 8
P = 128
MAGIC = 12582912.0  # 1.5 * 2^23: float + MAGIC - MAGIC == round-to-nearest-int
QS = 16384.0  # 2^14 logit quantization scale

# (name, S, rows_per_core, dect_size, [tile widths K_t])
#   P * sum(K_t) >= rows*S*3 (pad). Processing order large->small: the K=8/19
#   starter tiles shorten the DMA-gated ramp; small-last keeps the drain short.
LEVELS = [
    ("large", 416, 52, 5, [8, 19, 78, 78, 78, 78, 78, 78, 12]),
    ("middle", 208, 26, 4, [64, 64]),
    ("small", 104, 13, 3, [32]),
]
OUT_ORDER = ["small", "middle", "large"]

LAST_EXEC_NS = None
LAST_RESULTS = None

_prog_cache = {}


def _build_program():
    nc = bacc.Bacc(trn_type="TRN2")
    xins, csts, outs = {}, {}, {}
    for nm, S, rows, dect, KS in LEVELS:
        Ncap = P * sum(KS)
        W = 4 * sum(KS)  # per tile: gxy [P, 2K] then awh [P, 2K]
        xins[nm] = nc.dram_tensor(f"x_{nm}", [Ncap, 85], F32, kind="ExternalInput")
        csts[nm] = nc.dram_tensor(f"c_{nm}", [128, W], F32, kind="ExternalInput")
        outs[nm] = nc.dram_tensor(f"o_{nm}", [Ncap, 6], BF16, kind="ExternalOutput")
    # cols 0:80 = (79-c)/128 - 0.3125 ; col 80 = MAGIC (activation bias AP)
    iot = nc.dram_tensor("iota", [128, 81], F32, kind="ExternalInput")

    with tile.TileContext(nc) as tc, ExitStack() as ctx:
        const = ctx.enter_context(tc.tile_pool(name="const", bufs=1))
        pin_p = ctx.enter_context(tc.tile_pool(name="pin", bufs=3))
        qt_p = ctx.enter_context(tc.tile_pool(name="qtp", bufs=2))
        km_p = ctx.enter_context(tc.tile_pool(name="kmp", bufs=2))
        sml_p = ctx.enter_context(tc.tile_pool(name="sml", bufs=3))
        out_p = ctx.enter_context(tc.tile_pool(name="outp", bufs=3))

        it = const.tile([128, 81], F32)
        nc.sync.dma_start(it[:], iot[:])
        ctiles = {}
        for nm, S, rows, dect, KS in LEVELS:
            ct = const.tile([128, 4 * sum(KS)], F32)
            nc.sync.dma_start(ct[:], csts[nm][:])
            ctiles[nm] = ct

        magic_ap = it[0:128, 80:81]
        for nm, S, rows, dect, KS in LEVELS:
            inv = float(np.float32(1.0 / S))
            ct = ctiles[nm]
            off = 0
            coff = 0
            for K in KS:
                # tile t covers cells [P*off, P*off + P*K), partition p owns
                # K consecutive cells starting at P*off + p*K
                xv = xins[nm][P * off : P * off + P * K, :].rearrange(
                    "(p k) c -> p (k c)", k=K
                )
                ov = outs[nm][P * off : P * off + P * K, :].rearrange(
                    "(p k) c -> p (k c)", k=K
                )
                gxy_v = ct[0:P, coff : coff + 2 * K].rearrange("p (k c) -> p k c", c=2)
                awh_v = ct[0:P, coff + 2 * K : coff + 4 * K].rearrange(
                    "p (k c) -> p k c", c=2
                )
                iota_b = (
                    it[0:P, 0:80]
                    .rearrange("p (o c) -> p o c", o=1)
                    .broadcast_to([P, K, 80])
                )

                pin = pin_p.tile([P, K * 85], F32, tag="pin")
                nc.sync.dma_start(pin[:], xv)
                pv = pin[:].rearrange("p (k c) -> p k c", c=85)

                ot = out_p.tile([P, K * 6], BF16, tag="out")
                ovv = ot[:].rearrange("p (k c) -> p k c", c=6)

                # ACT: magic-round first (it gates DVE's big ops), then
                # exp(-txy) for sigmoid, exp(twh), conf copy
                qt = qt_p.tile([P, K * 80], F32, tag="qt")
                qv = qt[:].rearrange("p (k c) -> p k c", c=80)
                nc.scalar.activation(
                    qv, pv[:, :, 5:85], Act.Identity, bias=magic_ap, scale=QS
                )
                es = sml_p.tile([P, K * 2], F32, tag="es")
                esv = es[:].rearrange("p (k c) -> p k c", c=2)
                nc.scalar.activation(esv, pv[:, :, 1:3], Act.Exp, scale=-1.0)
                sp = sml_p.tile([P, K * 2], F32, tag="sp")
                spv = sp[:].rearrange("p (k c) -> p k c", c=2)
                nc.scalar.activation(spv, esv, Act.Identity, bias=1.0)
                ext = sml_p.tile([P, K * 2], F32, tag="exp")
                exv = ext[:].rearrange("p (k c) -> p k c", c=2)
                nc.scalar.activation(exv, pv[:, :, 3:5], Act.Exp)
                nc.scalar.copy(ovv[:, :, 5:6], pv[:, :, 0:1])

                # DVE: key = (t - M) + iota'   (in place over qt)
                nc.vector.scalar_tensor_tensor(qv, qv, -MAGIC, iota_b, Alu.add, Alu.add)

                # DVE: packed max over the 80 classes in one reduce
                km = km_p.tile([P, K], F32, tag="km")
                nc.vector.tensor_reduce(km[:], qv, axis=AxX, op=Alu.max)

                # DVE: t2 = M + r* (fp32 write rounds); e = (t2 - M) - kmax;
                # lix = 128*e + 39 (exact int)
                t2 = sml_p.tile([P, K], F32, tag="t2")
                nc.vector.tensor_scalar(t2[:], km[:], 1.0, MAGIC, Alu.mult, Alu.add)
                e = sml_p.tile([P, K], F32, tag="e")
                nc.vector.scalar_tensor_tensor(
                    e[:], t2[:], -MAGIC, km[:], Alu.add, Alu.subtract
                )
                ev = e[:].rearrange("p (k c) -> p k c", c=1)
                nc.vector.tensor_scalar(
                    ovv[:, :, 4:5], ev, 128.0, 39.0, Alu.mult, Alu.add
                )

                # DVE: sigmoid = 1/(1+exp(-x)); (x,y) = sig*inv + (gx,gy);
                # (w,h) = exp * (aw,ah); mask = conf > 0.5; zero failing rows
                sg = sml_p.tile([P, K * 2], F32, tag="sg")
                nc.vector.reciprocal_approx_fast(sg[:], sp[:])
                sgv = sg[:].rearrange("p (k c) -> p k c", c=2)
                nc.vector.scalar_tensor_tensor(
                    ovv[:, :, 0:2], sgv, inv, gxy_v, Alu.mult, Alu.add
                )
                nc.vector.scalar_tensor_tensor(
                    ovv[:, :, 2:4], exv, 1.0, awh_v, Alu.mult, Alu.mult
                )
                mk = sml_p.tile([P, K], BF16, tag="mk")
                nc.vector.tensor_scalar(mk[:], pv[:, :, 0], 0.5, None, Alu.is_gt)
                mk_b = mk[:].rearrange("p (k o) -> p k o", o=1).broadcast_to([P, K, 6])
                nc.vector.scalar_tensor_tensor(ovv, ovv, 1.0, mk_b, Alu.mult, Alu.mult)

                nc.sync.dma_start(ov, ot[:])
                off += K
                coff += 4 * K
    nc.compile()
    return nc


def _get_program():
    if "nc" not in _prog_cache:
        _prog_cache["nc"] = _build_program()
    return _prog_cache["nc"]


def _make_cst(core, ps, S, rows, dect, KS):
    """Per-level const table [128, 4*sum(K)]: per tile, gxy [P,2K] | awh [P,2K]."""
    inv = np.float32(1.0 / S)
    anc = (ps[dect] / np.float32(416.0)).astype(np.float32)  # [3, 2]
    cells = rows * S * 3
    base = core * cells
    cst = np.zeros((128, 4 * sum(KS)), np.float32)
    off = 0
    coff = 0
    p_idx = np.arange(P)[:, None]
    for K in KS:
        g = base + np.minimum(P * off + p_idx * K + np.arange(K)[None, :], cells - 1)
        i = g // (S * 3)
        j = (g % (S * 3)) // 3
        aa = g % 3
        blk = np.empty((P, K, 2), np.float32)
        blk[..., 0] = i.astype(np.float32) * inv
        blk[..., 1] = j.astype(np.float32) * inv
        cst[:P, coff : coff + 2 * K] = blk.reshape(P, 2 * K)
        blk[..., 0] = anc[aa, 0]
        blk[..., 1] = anc[aa, 1]
        cst[:P, coff + 2 * K : coff + 4 * K] = blk.reshape(P, 2 * K)
        off += K
        coff += 4 * K
    return cst


def _make_in_maps(small, middle, large, pre_scale):
    arrs = {"small": small, "middle": middle, "large": large}
    ps = np.asarray(pre_scale, dtype=np.float32)
    iota = np.empty((128, 81), np.float32)
    iota[:, 0:80] = (79.0 - np.arange(80)).astype(np.float32) / np.float32(
        128.0
    ) - np.float32(0.3125)
    iota[:, 80] = MAGIC
    in_maps = []
    for c in range(N_CORES):
        m = {"iota": iota}
        for nm, S, rows, dect, KS in LEVELS:
            cells = rows * S * 3
            Ncap = P * sum(KS)
            x = np.asarray(arrs[nm][c * rows : (c + 1) * rows], dtype=np.float32)
            xp = np.zeros((Ncap, 85), np.float32)
            xp[:cells] = x.reshape(cells, 85)
            m[f"x_{nm}"] = xp
            m[f"c_{nm}"] = _make_cst(c, ps, S, rows, dect, KS)
        in_maps.append(m)
    return in_maps


def kernel(small, middle, large, pre_scale):
    global LAST_EXEC_NS, LAST_RESULTS
    small = np.asarray(small, dtype=np.float32)
    middle = np.asarray(middle, dtype=np.float32)
    large = np.asarray(large, dtype=np.float32)
    in_maps = _make_in_maps(small, middle, large, pre_scale)
    nc = _get_program()
    res = run_bass_kernel_spmd(nc, in_maps, list(range(N_CORES)))
    LAST_EXEC_NS = res.exec_time_ns
    LAST_RESULTS = res
    by_name = {lv[0]: lv for lv in LEVELS}
    chunks = []
    for nm in OUT_ORDER:
        nm, S, rows, dect, KS = by_name[nm]
        cells = rows * S * 3
        for c in range(N_CORES):
            o = np.asarray(res.results[c][f"o_{nm}"])[:cells]
            chunks.append(o.astype(np.float32))
    return np.concatenate(chunks, axis=0)


# revision 18
# speedup vs baseline: 1.0201x; 1.0158x over previous
"""YOLO-head decode (nms_detection) Bass kernel for 8 trn2 NeuronCores.

Reference computation per pyramid level p [S, S, 3, 85]:
  conf = p[...,0]
  x = (sigmoid(p[...,1]) + i) / S        (i = index along FIRST spatial axis)
  y = (sigmoid(p[...,2]) + j) / S
  w = exp(p[...,3]) * anchor_w           (anchor = pre_scale[dect]/416)
  h = exp(p[...,4]) * anchor_h
  lix = argmax(p[...,5:85])  (first-max tie-break)
  row = [x,y,w,h,lix,conf] * (conf > 0.5)
Output = concat over levels of rows, [681408, 6].

Sharding: each level split along its leading spatial axis into 8 row-shards
(104->13, 208->26, 416->52 rows per core). Decode is elementwise per cell, so
cores are fully independent; host concatenates the per-core outputs.

Layout: cells on partitions (always all 128), K cells per partition per tile
with a per-level list of tile widths. The large level starts with K=8/K=19
tiles so the first DVE work is gated by only ~350KB of DMA (short pipeline
ramp), then runs K=78 tiles to amortize per-instruction overheads. Levels are
padded (pad cells are zeros -> conf 0 -> masked; host drops them). Grid x/y
offsets and anchors are per-(tile, partition, k) host-computed const tables.

argmax via value/index packing so one max-reduce yields both:
  ACT:  t   = Identity(v * 2^14 + 1.5*2^23)      -> fp32 magic-round, t = M + r
  DVE:  key = (t - M) + ((79-c)/128 - 0.3125)    -> r + (m-40)/128, m = 79-c
  DVE:  kmax = reduce_max(key)  (single InstTensorReduce over the 80 classes)
  DVE:  t2  = kmax + M (fp32 write rounds) ; e = (t2 - M) - kmax = -(m*-40)/128
  DVE:  lix = 128*e + 39 = c*
All steps are exact in fp32 for |v| < 8 (keys are integers r plus exact
multiples of 1/128, |r| <= 2^17), so the only deviation from jnp.argmax is
the 2^-14 quantization of logits (ties broken toward the first index, same
as argmax; measured rel-err contribution ~3.7e-3, well under the 2e-2 gate).

sigmoid is computed as 1/(1+exp(-x)) (ACT Exp with scale=-1 + Identity(+1),
DVE reciprocal_approx_fast) because no ACT table set holds both sigmoid and
exp -- a native Sigmoid would force two 1.3us table reloads per tile.

Outputs are written bf16 (halves output DMA; x/y/w/h/conf tolerate the 2^-9
rounding, lix <= 79 is exact in bf16) and converted to fp32 on host.
"""

import os
import sys
from contextlib import ExitStack

import numpy as np

for _p in ("/root/.axon_site/_ro/trn_rl_repo", "/opt/trn_rl_repo"):
    if os.path.isdir(_p) and _p not in sys.path:
        sys.path.append(_p)

import concourse.bacc as bacc
import concourse.bass as bass
import concourse.tile as tile
import concourse.mybir as mybir
from concourse.bass_utils import run_bass_kernel_spmd

F32 = mybir.dt.float32
BF16 = mybir.dt.bfloat16
Alu = mybir.AluOpType
Act = mybir.ActivationFunctionType
AxX = mybir.AxisListType.X

N_CORES = 8
P = 128
MAGIC = 12582912.0  # 1.5 * 2^23: float + MAGIC - MAGIC == round-to-nearest-int
QS = 16384.0  # 2^14 logit quantization scale

# (name, S, rows_per_core, dect_size, [tile widths K_t])
#   P * sum(K_t) >= rows*S*3 (pad). Processing order large->small: the K=8/19
#   starter tiles shorten the DMA-gated ramp; small-last keeps the drain short.
LEVELS = [
    ("large", 416, 52, 5, [8, 16, 28, 44, 64, 78, 78, 78, 78, 35]),
    ("middle", 208, 26, 4, [64, 64]),
    ("small", 104, 13, 3, [32]),
]
OUT_ORDER = ["small", "middle", "large"]

LAST_EXEC_NS = None
LAST_RESULTS = None

_prog_cache = {}


def _build_program():
    nc = bacc.Bacc(trn_type="TRN2")
    xins, csts, outs = {}, {}, {}
    for nm, S, rows, dect, KS in LEVELS:
        Ncap = P * sum(KS)
        W = 4 * sum(KS)  # per tile: gxy [P, 2K] then awh [P, 2K]
        xins[nm] = nc.dram_tensor(f"x_{nm}", [Ncap, 85], F32, kind="ExternalInput")
        csts[nm] = nc.dram_tensor(f"c_{nm}", [128, W], F32, kind="ExternalInput")
        outs[nm] = nc.dram_tensor(f"o_{nm}", [Ncap, 6], BF16, kind="ExternalOutput")
    # cols 0:80 = (79-c)/128 - 0.3125 ; col 80 = MAGIC (activation bias AP)
    iot = nc.dram_tensor("iota", [128, 81], F32, kind="ExternalInput")

    with tile.TileContext(nc) as tc, ExitStack() as ctx:
        const = ctx.enter_context(tc.tile_pool(name="const", bufs=1))
        pin_p = ctx.enter_context(tc.tile_pool(name="pin", bufs=3))
        qt_p = ctx.enter_context(tc.tile_pool(name="qtp", bufs=2))
        km_p = ctx.enter_context(tc.tile_pool(name="kmp", bufs=2))
        sml_p = ctx.enter_context(tc.tile_pool(name="sml", bufs=3))
        out_p = ctx.enter_context(tc.tile_pool(name="outp", bufs=3))

        it = const.tile([128, 81], F32)
        ctiles = {
            nm: const.tile([128, 4 * sum(KS)], F32, name=f"ct_{nm}")
            for nm, *_r, KS in LEVELS
        }
        consts_issued = False

        magic_ap = it[0:128, 80:81]
        for nm, S, rows, dect, KS in LEVELS:
            inv = float(np.float32(1.0 / S))
            ct = ctiles[nm]
            off = 0
            coff = 0
            for K in KS:
                # tile t covers cells [P*off, P*off + P*K), partition p owns
                # K consecutive cells starting at P*off + p*K
                xv = xins[nm][P * off : P * off + P * K, :].rearrange(
                    "(p k) c -> p (k c)", k=K
                )
                ov = outs[nm][P * off : P * off + P * K, :].rearrange(
                    "(p k) c -> p (k c)", k=K
                )
                gxy_v = ct[0:P, coff : coff + 2 * K].rearrange("p (k c) -> p k c", c=2)
                awh_v = ct[0:P, coff + 2 * K : coff + 4 * K].rearrange(
                    "p (k c) -> p k c", c=2
                )
                iota_b = (
                    it[0:P, 0:80]
                    .rearrange("p (o c) -> p o c", o=1)
                    .broadcast_to([P, K, 80])
                )

                pin = pin_p.tile([P, K * 85], F32, tag="pin")
                nc.sync.dma_start(pin[:], xv)
                if not consts_issued:
                    # const DMAs queue behind the first input tile so the
                    # pipeline's first quantize isn't delayed by them
                    nc.sync.dma_start(it[:], iot[:])
                    for _nm, *_r2, _KS in LEVELS:
                        nc.sync.dma_start(ctiles[_nm][:], csts[_nm][:])
                    consts_issued = True
                pv = pin[:].rearrange("p (k c) -> p k c", c=85)

                ot = out_p.tile([P, K * 6], BF16, tag="out")
                ovv = ot[:].rearrange("p (k c) -> p k c", c=6)

                # ACT: magic-round first (it gates DVE's big ops), then
                # exp(-txy) for sigmoid, exp(twh), conf copy
                qt = qt_p.tile([P, K * 80], F32, tag="qt")
                qv = qt[:].rearrange("p (k c) -> p k c", c=80)
                nc.scalar.activation(
                    qv, pv[:, :, 5:85], Act.Identity, bias=magic_ap, scale=QS
                )
                es = sml_p.tile([P, K * 2], F32, tag="es")
                esv = es[:].rearrange("p (k c) -> p k c", c=2)
                nc.scalar.activation(esv, pv[:, :, 1:3], Act.Exp, scale=-1.0)
                sp = sml_p.tile([P, K * 2], F32, tag="sp")
                spv = sp[:].rearrange("p (k c) -> p k c", c=2)
                nc.scalar.activation(spv, esv, Act.Identity, bias=1.0)
                ext = sml_p.tile([P, K * 2], F32, tag="exp")
                exv = ext[:].rearrange("p (k c) -> p k c", c=2)
                nc.scalar.activation(exv, pv[:, :, 3:5], Act.Exp)
                nc.scalar.copy(ovv[:, :, 5:6], pv[:, :, 0:1])

                # DVE: key = (t - M) + iota'   (in place over qt)
                nc.vector.scalar_tensor_tensor(qv, qv, -MAGIC, iota_b, Alu.add, Alu.add)

                # DVE: packed max over the 80 classes in one reduce
                km = km_p.tile([P, K], F32, tag="km")
                nc.vector.tensor_reduce(km[:], qv, axis=AxX, op=Alu.max)

                # DVE: t2 = M + r* (fp32 write rounds); e = (t2 - M) - kmax;
                # lix = 128*e + 39 (exact int)
                t2 = sml_p.tile([P, K], F32, tag="t2")
                nc.vector.tensor_scalar(t2[:], km[:], 1.0, MAGIC, Alu.mult, Alu.add)
                e = sml_p.tile([P, K], F32, tag="e")
                nc.vector.scalar_tensor_tensor(
                    e[:], t2[:], -MAGIC, km[:], Alu.add, Alu.subtract
                )
                ev = e[:].rearrange("p (k c) -> p k c", c=1)
                nc.vector.tensor_scalar(
                    ovv[:, :, 4:5], ev, 128.0, 39.0, Alu.mult, Alu.add
                )

                # DVE: sigmoid = 1/(1+exp(-x)); (x,y) = sig*inv + (gx,gy);
                # (w,h) = exp * (aw,ah); mask = conf > 0.5; zero failing rows
                sg = sml_p.tile([P, K * 2], F32, tag="sg")
                nc.vector.reciprocal_approx_fast(sg[:], sp[:])
                sgv = sg[:].rearrange("p (k c) -> p k c", c=2)
                nc.vector.scalar_tensor_tensor(
                    ovv[:, :, 0:2], sgv, inv, gxy_v, Alu.mult, Alu.add
                )
                nc.vector.scalar_tensor_tensor(
                    ovv[:, :, 2:4], exv, 1.0, awh_v, Alu.mult, Alu.mult
                )
                mk = sml_p.tile([P, K], BF16, tag="mk")
                nc.vector.tensor_scalar(mk[:], pv[:, :, 0], 0.5, None, Alu.is_gt)
                mk_b = mk[:].rearrange("p (k o) -> p k o", o=1).broadcast_to([P, K, 6])
                nc.vector.scalar_tensor_tensor(ovv, ovv, 1.0, mk_b, Alu.mult, Alu.mult)

                nc.sync.dma_start(ov, ot[:])
                off += K
                coff += 4 * K
    nc.compile()
    return nc


def _get_program():
    if "nc" not in _prog_cache:
        _prog_cache["nc"] = _build_program()
    return _prog_cache["nc"]


def _make_cst(core, ps, S, rows, dect, KS):
    """Per-level const table [128, 4*sum(K)]: per tile, gxy [P,2K] | awh [P,2K]."""
    inv = np.float32(1.0 / S)
    anc = (ps[dect] / np.float32(416.0)).astype(np.float32)  # [3, 2]
    cells = rows * S * 3
    base = core * cells
    cst = np.zeros((128, 4 * sum(KS)), np.float32)
    off = 0
    coff = 0
    p_idx = np.arange(P)[:, None]
    for K in KS:
        g = base + np.minimum(P * off + p_idx * K + np.arange(K)[None, :], cells - 1)
        i = g // (S * 3)
        j = (g % (S * 3)) // 3
        aa = g % 3
        blk = np.empty((P, K, 2), np.float32)
        blk[..., 0] = i.astype(np.float32) * inv
        blk[..., 1] = j.astype(np.float32) * inv
        cst[:P, coff : coff + 2 * K] = blk.reshape(P, 2 * K)
        blk[..., 0] = anc[aa, 0]
        blk[..., 1] = anc[aa, 1]
        cst[:P, coff + 2 * K : coff + 4 * K] = blk.reshape(P, 2 * K)
        off += K
        coff += 4 * K
    return cst


def _make_in_maps(small, middle, large, pre_scale):
    arrs = {"small": small, "middle": middle, "large": large}
    ps = np.asarray(pre_scale, dtype=np.float32)
    iota = np.empty((128, 81), np.float32)
    iota[:, 0:80] = (79.0 - np.arange(80)).astype(np.float32) / np.float32(
        128.0
    ) - np.float32(0.3125)
    iota[:, 80] = MAGIC
    in_maps = []
    for c in range(N_CORES):
        m = {"iota": iota}
        for nm, S, rows, dect, KS in LEVELS:
            cells = rows * S * 3
            Ncap = P * sum(KS)
            x = np.asarray(arrs[nm][c * rows : (c + 1) * rows], dtype=np.float32)
            xp = np.zeros((Ncap, 85), np.float32)
            xp[:cells] = x.reshape(cells, 85)
            m[f"x_{nm}"] = xp
            m[f"c_{nm}"] = _make_cst(c, ps, S, rows, dect, KS)
        in_maps.append(m)
    return in_maps


def kernel(small, middle, large, pre_scale):
    global LAST_EXEC_NS, LAST_RESULTS
    small = np.asarray(small, dtype=np.float32)
    middle = np.asarray(middle, dtype=np.float32)
    large = np.asarray(large, dtype=np.float32)
    in_maps = _make_in_maps(small, middle, large, pre_scale)
    nc = _get_program()
    res = run_bass_kernel_spmd(nc, in_maps, list(range(N_CORES)))
    LAST_EXEC_NS = res.exec_time_ns
    LAST_RESULTS = res
    by_name = {lv[0]: lv for lv in LEVELS}
    chunks = []
    for nm in OUT_ORDER:
        nm, S, rows, dect, KS = by_name[nm]
        cells = rows * S * 3
        for c in range(N_CORES):
            o = np.asarray(res.results[c][f"o_{nm}"])[:cells]
            chunks.append(o.astype(np.float32))
    return np.concatenate(chunks, axis=0)


# revision 19
# speedup vs baseline: 1.0246x; 1.0045x over previous
"""YOLO-head decode (nms_detection) Bass kernel for 8 trn2 NeuronCores.

Reference computation per pyramid level p [S, S, 3, 85]:
  conf = p[...,0]
  x = (sigmoid(p[...,1]) + i) / S        (i = index along FIRST spatial axis)
  y = (sigmoid(p[...,2]) + j) / S
  w = exp(p[...,3]) * anchor_w           (anchor = pre_scale[dect]/416)
  h = exp(p[...,4]) * anchor_h
  lix = argmax(p[...,5:85])  (first-max tie-break)
  row = [x,y,w,h,lix,conf] * (conf > 0.5)
Output = concat over levels of rows, [681408, 6].

Sharding: each level split along its leading spatial axis into 8 row-shards
(104->13, 208->26, 416->52 rows per core). Decode is elementwise per cell, so
cores are fully independent; host concatenates the per-core outputs.

Layout: cells on partitions (always all 128), K cells per partition per tile
with a per-level list of tile widths. The large level starts with K=8/K=19
tiles so the first DVE work is gated by only ~350KB of DMA (short pipeline
ramp), then runs K=78 tiles to amortize per-instruction overheads. Levels are
padded (pad cells are zeros -> conf 0 -> masked; host drops them). Grid x/y
offsets and anchors are per-(tile, partition, k) host-computed const tables.

argmax via value/index packing so one max-reduce yields both:
  ACT:  t   = Identity(v * 2^14 + 1.5*2^23)      -> fp32 magic-round, t = M + r
  DVE:  key = (t - M) + ((79-c)/128 - 0.3125)    -> r + (m-40)/128, m = 79-c
  DVE:  kmax = reduce_max(key)  (single InstTensorReduce over the 80 classes)
  DVE:  t2  = kmax + M (fp32 write rounds) ; e = (t2 - M) - kmax = -(m*-40)/128
  DVE:  lix = 128*e + 39 = c*
All steps are exact in fp32 for |v| < 8 (keys are integers r plus exact
multiples of 1/128, |r| <= 2^17), so the only deviation from jnp.argmax is
the 2^-14 quantization of logits (ties broken toward the first index, same
as argmax; measured rel-err contribution ~3.7e-3, well under the 2e-2 gate).

sigmoid is computed as 1/(1+exp(-x)) (ACT Exp with scale=-1 + Identity(+1),
DVE reciprocal_approx_fast) because no ACT table set holds both sigmoid and
exp -- a native Sigmoid would force two 1.3us table reloads per tile.

Outputs are written bf16 (halves output DMA; x/y/w/h/conf tolerate the 2^-9
rounding, lix <= 79 is exact in bf16) and converted to fp32 on host.
"""

import os
import sys
from contextlib import ExitStack

import numpy as np

for _p in ("/root/.axon_site/_ro/trn_rl_repo", "/opt/trn_rl_repo"):
    if os.path.isdir(_p) and _p not in sys.path:
        sys.path.append(_p)

import concourse.bacc as bacc
import concourse.bass as bass
import concourse.tile as tile
import concourse.mybir as mybir
from concourse.bass_utils import run_bass_kernel_spmd

F32 = mybir.dt.float32
BF16 = mybir.dt.bfloat16
Alu = mybir.AluOpType
Act = mybir.ActivationFunctionType
AxX = mybir.AxisListType.X

N_CORES = 8
P = 128
MAGIC = 12582912.0  # 1.5 * 2^23: float + MAGIC - MAGIC == round-to-nearest-int
QS = 16384.0  # 2^14 logit quantization scale

# (name, S, rows_per_core, dect_size, [tile widths K_t])
#   P * sum(K_t) >= rows*S*3 (pad). Processing order large->small: the K=8/19
#   starter tiles shorten the DMA-gated ramp; small-last keeps the drain short.
LEVELS = [
    ("large", 416, 52, 5, [8, 16, 28, 44, 64, 78, 78, 78, 78, 35]),
    ("middle", 208, 26, 4, [64, 64]),
    ("small", 104, 13, 3, [32]),
]
OUT_ORDER = ["small", "middle", "large"]

LAST_EXEC_NS = None
LAST_RESULTS = None

_prog_cache = {}


def _build_program():
    nc = bacc.Bacc(trn_type="TRN2")
    xins, csts, outs = {}, {}, {}
    for nm, S, rows, dect, KS in LEVELS:
        Ncap = P * sum(KS)
        W = 4 * sum(KS)  # per tile: gxy [P, 2K] then awh [P, 2K]
        xins[nm] = nc.dram_tensor(f"x_{nm}", [Ncap, 85], F32, kind="ExternalInput")
        csts[nm] = nc.dram_tensor(f"c_{nm}", [128, W], F32, kind="ExternalInput")
        outs[nm] = nc.dram_tensor(f"o_{nm}", [Ncap, 6], BF16, kind="ExternalOutput")
    # cols 0:80 = (79-c)/128 - 0.3125 ; col 80 = MAGIC (activation bias AP)
    iot = nc.dram_tensor("iota", [128, 81], F32, kind="ExternalInput")

    with tile.TileContext(nc) as tc, ExitStack() as ctx:
        const = ctx.enter_context(tc.tile_pool(name="const", bufs=1))
        pin_p = ctx.enter_context(tc.tile_pool(name="pin", bufs=3))
        qt_p = ctx.enter_context(tc.tile_pool(name="qtp", bufs=3))
        km_p = ctx.enter_context(tc.tile_pool(name="kmp", bufs=2))
        sml_p = ctx.enter_context(tc.tile_pool(name="sml", bufs=3))
        out_p = ctx.enter_context(tc.tile_pool(name="outp", bufs=4))

        it = const.tile([128, 81], F32)
        ctiles = {
            nm: const.tile([128, 4 * sum(KS)], F32, name=f"ct_{nm}")
            for nm, *_r, KS in LEVELS
        }
        consts_issued = False

        magic_ap = it[0:128, 80:81]
        for nm, S, rows, dect, KS in LEVELS:
            inv = float(np.float32(1.0 / S))
            ct = ctiles[nm]
            off = 0
            coff = 0
            for K in KS:
                # tile t covers cells [P*off, P*off + P*K), partition p owns
                # K consecutive cells starting at P*off + p*K
                xv = xins[nm][P * off : P * off + P * K, :].rearrange(
                    "(p k) c -> p (k c)", k=K
                )
                ov = outs[nm][P * off : P * off + P * K, :].rearrange(
                    "(p k) c -> p (k c)", k=K
                )
                gxy_v = ct[0:P, coff : coff + 2 * K].rearrange("p (k c) -> p k c", c=2)
                awh_v = ct[0:P, coff + 2 * K : coff + 4 * K].rearrange(
                    "p (k c) -> p k c", c=2
                )
                iota_b = (
                    it[0:P, 0:80]
                    .rearrange("p (o c) -> p o c", o=1)
                    .broadcast_to([P, K, 80])
                )

                pin = pin_p.tile([P, K * 85], F32, tag="pin")
                nc.sync.dma_start(pin[:], xv)
                if not consts_issued:
                    # const DMAs queue behind the first input tile so the
                    # pipeline's first quantize isn't delayed by them
                    nc.sync.dma_start(it[:], iot[:])
                    for _nm, *_r2, _KS in LEVELS:
                        nc.sync.dma_start(ctiles[_nm][:], csts[_nm][:])
                    consts_issued = True
                pv = pin[:].rearrange("p (k c) -> p k c", c=85)

                ot = out_p.tile([P, K * 6], BF16, tag="out")
                ovv = ot[:].rearrange("p (k c) -> p k c", c=6)

                # ACT: magic-round first (it gates DVE's big ops), then
                # exp(-txy) for sigmoid, exp(twh), conf copy
                qt = qt_p.tile([P, K * 80], F32, tag="qt")
                qv = qt[:].rearrange("p (k c) -> p k c", c=80)
                nc.scalar.activation(
                    qv, pv[:, :, 5:85], Act.Identity, bias=magic_ap, scale=QS
                )
                es = sml_p.tile([P, K * 2], F32, tag="es")
                esv = es[:].rearrange("p (k c) -> p k c", c=2)
                nc.scalar.activation(esv, pv[:, :, 1:3], Act.Exp, scale=-1.0)
                sp = sml_p.tile([P, K * 2], F32, tag="sp")
                spv = sp[:].rearrange("p (k c) -> p k c", c=2)
                nc.scalar.activation(spv, esv, Act.Identity, bias=1.0)
                ext = sml_p.tile([P, K * 2], F32, tag="exp")
                exv = ext[:].rearrange("p (k c) -> p k c", c=2)
                nc.scalar.activation(exv, pv[:, :, 3:5], Act.Exp)
                nc.scalar.copy(ovv[:, :, 5:6], pv[:, :, 0:1])

                # DVE: key = (t - M) + iota'   (in place over qt)
                nc.vector.scalar_tensor_tensor(qv, qv, -MAGIC, iota_b, Alu.add, Alu.add)

                # DVE: packed max over the 80 classes in one reduce
                km = km_p.tile([P, K], F32, tag="km")
                nc.vector.tensor_reduce(km[:], qv, axis=AxX, op=Alu.max)

                # DVE: t2 = M + r* (fp32 write rounds); e = (t2 - M) - kmax;
                # lix = 128*e + 39 (exact int)
                t2 = sml_p.tile([P, K], F32, tag="t2")
                nc.vector.tensor_scalar(t2[:], km[:], 1.0, MAGIC, Alu.mult, Alu.add)
                e = sml_p.tile([P, K], F32, tag="e")
                nc.vector.scalar_tensor_tensor(
                    e[:], t2[:], -MAGIC, km[:], Alu.add, Alu.subtract
                )
                ev = e[:].rearrange("p (k c) -> p k c", c=1)
                nc.vector.tensor_scalar(
                    ovv[:, :, 4:5], ev, 128.0, 39.0, Alu.mult, Alu.add
                )

                # DVE: sigmoid = 1/(1+exp(-x)); (x,y) = sig*inv + (gx,gy);
                # (w,h) = exp * (aw,ah); mask = conf > 0.5; zero failing rows
                sg = sml_p.tile([P, K * 2], F32, tag="sg")
                nc.vector.reciprocal_approx_fast(sg[:], sp[:])
                sgv = sg[:].rearrange("p (k c) -> p k c", c=2)
                nc.vector.scalar_tensor_tensor(
                    ovv[:, :, 0:2], sgv, inv, gxy_v, Alu.mult, Alu.add
                )
                nc.vector.scalar_tensor_tensor(
                    ovv[:, :, 2:4], exv, 1.0, awh_v, Alu.mult, Alu.mult
                )
                mk = sml_p.tile([P, K], BF16, tag="mk")
                nc.vector.tensor_scalar(mk[:], pv[:, :, 0], 0.5, None, Alu.is_gt)
                mk_b = mk[:].rearrange("p (k o) -> p k o", o=1).broadcast_to([P, K, 6])
                nc.vector.scalar_tensor_tensor(ovv, ovv, 1.0, mk_b, Alu.mult, Alu.mult)

                nc.sync.dma_start(ov, ot[:])
                off += K
                coff += 4 * K
    nc.compile()
    return nc


def _get_program():
    if "nc" not in _prog_cache:
        _prog_cache["nc"] = _build_program()
    return _prog_cache["nc"]


def _make_cst(core, ps, S, rows, dect, KS):
    """Per-level const table [128, 4*sum(K)]: per tile, gxy [P,2K] | awh [P,2K]."""
    inv = np.float32(1.0 / S)
    anc = (ps[dect] / np.float32(416.0)).astype(np.float32)  # [3, 2]
    cells = rows * S * 3
    base = core * cells
    cst = np.zeros((128, 4 * sum(KS)), np.float32)
    off = 0
    coff = 0
    p_idx = np.arange(P)[:, None]
    for K in KS:
        g = base + np.minimum(P * off + p_idx * K + np.arange(K)[None, :], cells - 1)
        i = g // (S * 3)
        j = (g % (S * 3)) // 3
        aa = g % 3
        blk = np.empty((P, K, 2), np.float32)
        blk[..., 0] = i.astype(np.float32) * inv
        blk[..., 1] = j.astype(np.float32) * inv
        cst[:P, coff : coff + 2 * K] = blk.reshape(P, 2 * K)
        blk[..., 0] = anc[aa, 0]
        blk[..., 1] = anc[aa, 1]
        cst[:P, coff + 2 * K : coff + 4 * K] = blk.reshape(P, 2 * K)
        off += K
        coff += 4 * K
    return cst


def _make_in_maps(small, middle, large, pre_scale):
    arrs = {"small": small, "middle": middle, "large": large}
    ps = np.asarray(pre_scale, dtype=np.float32)
    iota = np.empty((128, 81), np.float32)
    iota[:, 0:80] = (79.0 - np.arange(80)).astype(np.float32) / np.float32(
        128.0
    ) - np.float32(0.3125)
    iota[:, 80] = MAGIC
    in_maps = []
    for c in range(N_CORES):
        m = {"iota": iota}
        for nm, S, rows, dect, KS in LEVELS:
            cells = rows * S * 3
            Ncap = P * sum(KS)
            x = np.asarray(arrs[nm][c * rows : (c + 1) * rows], dtype=np.float32)
            xp = np.zeros((Ncap, 85), np.float32)
            xp[:cells] = x.reshape(cells, 85)
            m[f"x_{nm}"] = xp
            m[f"c_{nm}"] = _make_cst(c, ps, S, rows, dect, KS)
        in_maps.append(m)
    return in_maps


def kernel(small, middle, large, pre_scale):
    global LAST_EXEC_NS, LAST_RESULTS
    small = np.asarray(small, dtype=np.float32)
    middle = np.asarray(middle, dtype=np.float32)
    large = np.asarray(large, dtype=np.float32)
    in_maps = _make_in_maps(small, middle, large, pre_scale)
    nc = _get_program()
    res = run_bass_kernel_spmd(nc, in_maps, list(range(N_CORES)))
    LAST_EXEC_NS = res.exec_time_ns
    LAST_RESULTS = res
    by_name = {lv[0]: lv for lv in LEVELS}
    chunks = []
    for nm in OUT_ORDER:
        nm, S, rows, dect, KS = by_name[nm]
        cells = rows * S * 3
        for c in range(N_CORES):
            o = np.asarray(res.results[c][f"o_{nm}"])[:cells]
            chunks.append(o.astype(np.float32))
    return np.concatenate(chunks, axis=0)
